# revision 28
# baseline (speedup 1.0000x reference)
"""Multi-head causal attention (B=4, S=2048, D=1024, H=16) on 8 TRN2 NeuronCores.

Sharding: core c -> (batch c//2, head-group c%2 of 8 heads = 512 d_model cols).
Each core:
  - projects Q/K/V for its head slice (bf16 matmuls, fp32 accum)
  - causal attention for its 8 heads over the full sequence, computed with
    scores transposed ([keys, q]) so exp(scores)^T feeds the A@V matmul as the
    moving operand; V is augmented with a ones column so softmax sums fall out
    of the same matmul
  - partial out-projection ctx^T @ Wo[rows-of-its-heads]  (no bias)
Host: out[b] = partial[2b] + partial[2b+1] + bo.
"""

import numpy as np
import ml_dtypes
from contextlib import ExitStack

import concourse.bass as bass
import concourse.tile as tile
from concourse import bacc, mybir
from concourse.bass_utils import run_bass_kernel_spmd

F32 = mybir.dt.float32
BF16 = mybir.dt.bfloat16
EXP = mybir.ActivationFunctionType.Exp

N_CORES = 8
S = 2048          # sequence length
D = 1024          # d_model
HL = 8            # heads per core
HD = 64           # head dim
DL = HL * HD      # local d_model slice = 512
SCALE = 1.0 / 8.0  # 1/sqrt(HD)

NQT = S // 128    # 16 q/seq tiles of 128
NQC = S // 512    # 4 q chunks of 512
NKT = S // 128    # 16 key tiles of 128
NDT = D // 128    # 8 d_model(in) tiles
NMT = DL // 128   # 4 local dout tiles (head pairs)
G = 2             # key-tiles per scores/exp group (2 PSUM banks)

_compiled = None  # cached (nc,) so repeated kernel() calls skip rebuild


def _build():
    nc = bacc.Bacc("TRN2", target_bir_lowering=False, debug=False,
                   num_devices=N_CORES)

    xq_ap = nc.dram_tensor("xqt", [D, S], BF16, kind="ExternalInput").ap()
    xk_ap = nc.dram_tensor("xkt", [D, S], BF16, kind="ExternalInput").ap()
    xv_ap = nc.dram_tensor("xvt", [D, S], BF16, kind="ExternalInput").ap()
    wq_ap = nc.dram_tensor("wq", [D, DL], BF16, kind="ExternalInput").ap()
    wk_ap = nc.dram_tensor("wk", [D, DL], BF16, kind="ExternalInput").ap()
    wv_ap = nc.dram_tensor("wv", [D, DL], BF16, kind="ExternalInput").ap()
    bq_ap = nc.dram_tensor("bq", [DL, 1], F32, kind="ExternalInput").ap()
    bk_ap = nc.dram_tensor("bk", [DL, 1], F32, kind="ExternalInput").ap()
    bvb_ap = nc.dram_tensor("bvb", [128, DL], F32, kind="ExternalInput").ap()
    wo_ap = nc.dram_tensor("wo", [DL, D], BF16, kind="ExternalInput").ap()
    out_ap = nc.dram_tensor("out", [S, D], F32, kind="ExternalOutput").ap()

    with tile.TileContext(nc) as tc, ExitStack() as ctx:
        wpool = ctx.enter_context(tc.tile_pool(name="weights", bufs=1))
        xt_pool = ctx.enter_context(tc.tile_pool(name="xt", bufs=56))
        qkv_pool = ctx.enter_context(tc.tile_pool(name="qkv", bufs=1))
        exp_pool = ctx.enter_context(tc.tile_pool(name="expt", bufs=6))
        norm_pool = ctx.enter_context(tc.tile_pool(name="norm", bufs=4))
        outst_pool = ctx.enter_context(tc.tile_pool(name="outst", bufs=3))
        psum_big = ctx.enter_context(tc.tile_pool(name="ps_big", bufs=3, space="PSUM"))
        psum_ctx = ctx.enter_context(tc.tile_pool(name="ps_ctx", bufs=2, space="PSUM"))

        # ---- weights / biases (already bf16 in DRAM) ----
        def load_w(dram, shape, nm):
            t16 = wpool.tile(shape, BF16, tag=nm, name=nm)
            nc.sync.dma_start(t16[:], dram)
            return t16

        wq_sb = [load_w(wq_ap[128 * d:128 * (d + 1), :], [128, DL], f"wq{d}") for d in range(NDT)]
        wk_sb = [load_w(wk_ap[128 * d:128 * (d + 1), :], [128, DL], f"wk{d}") for d in range(NDT)]
        wv_sb = [load_w(wv_ap[128 * d:128 * (d + 1), :], [128, DL], f"wv{d}") for d in range(NDT)]
        wo_sb = [load_w(wo_ap[128 * d:128 * (d + 1), :], [128, D], f"wo{d}") for d in range(NMT)]

        bq_sb = wpool.tile([128, NMT], F32, tag="bq")
        bk_sb = wpool.tile([128, NMT], F32, tag="bk")
        for m in range(NMT):
            nc.sync.dma_start(bq_sb[:, m:m + 1], bq_ap[128 * m:128 * (m + 1), :])
            nc.sync.dma_start(bk_sb[:, m:m + 1], bk_ap[128 * m:128 * (m + 1), :])
        bvb_sb = wpool.tile([128, DL], F32, tag="bvb")
        nc.sync.dma_start(bvb_sb[:], bvb_ap[:])

        # ---- x^T chunk load (host pre-transposed + pre-cast bf16) ----
        # 8 tiles [128 din, 512 seq] per (input, chunk)
        def load_xt_chunk(x_ap, qc, nm):
            xt = []
            for d in range(NDT):
                t = xt_pool.tile([128, 512], BF16, tag="xt", name=f"{nm}xt{qc}_{d}")
                nc.sync.dma_start(
                    t[:], x_ap[128 * d:128 * (d + 1), 512 * qc:512 * (qc + 1)])
                xt.append(t)
            return xt

        # qT/kT: [DL, S] bf16 stored as NMT tiles [128, S]
        qT = [qkv_pool.tile([128, S], BF16, tag=f"qT{m}", name=f"qT{m}") for m in range(NMT)]
        kT = [qkv_pool.tile([128, S], BF16, tag=f"kT{m}", name=f"kT{m}") for m in range(NMT)]

        def proj_chunk(xt, w_sb, b_sb, res, qc, m):
            ps = psum_big.tile([128, 512], F32, tag="big", name="ps")
            for d in range(NDT):
                nc.tensor.matmul(
                    ps[:], w_sb[d][:, 128 * m:128 * (m + 1)],
                    xt[d][:],
                    start=(d == 0), stop=(d == NDT - 1))
            nc.vector.tensor_scalar_add(
                res[m][:, 512 * qc:512 * (qc + 1)], ps[:],
                b_sb[:, m:m + 1])

        # v_aug: per seq-tile [128, HL*(HD+1)] bf16; per head 64 v cols + ones col
        v_aug = [None] * NQT

        def v_chunk(xt, qc, sti):
            st = 4 * qc + sti
            va = qkv_pool.tile([128, HL * (HD + 1)], BF16, tag=f"va{st}",
                               name=f"va{st}")
            nc.vector.memset(va[:], 1.0)
            ps = psum_big.tile([128, DL], F32, tag="big", name="ps")
            for d in range(NDT):
                nc.tensor.matmul(ps[:], xt[d][:, 128 * sti:128 * (sti + 1)],
                                 wv_sb[d][:], start=(d == 0), stop=(d == NDT - 1))
            va3 = va[:].rearrange("p (h c) -> p h c", h=HL)[:, :, 0:HD]
            nc.vector.tensor_add(
                va3,
                ps[:].rearrange("p (h c) -> p h c", h=HL),
                bvb_sb[:].rearrange("p (h c) -> p h c", h=HL))
            v_aug[st] = va

        # ---- attention + out projection, per q-chunk ----
        # ctxT: per head-pair tile [128, S] bf16 (rows 64*(h%2) for head h)
        ctxT = [qkv_pool.tile([128, S], BF16, tag=f"ctxT{m}", name=f"ctxT{m}") for m in range(NMT)]

        def emit_outproj(qt):
            ot = outst_pool.tile([128, 1024], F32, tag="ot", name="ot")
            for n in range(2):
                po_ps = psum_big.tile([128, 512], F32, tag="big", name="po_ps")
                for d in range(NMT):
                    nc.tensor.matmul(
                        po_ps[:],
                        ctxT[d][:, 128 * qt:128 * (qt + 1)],
                        wo_sb[d][:, 512 * n:512 * (n + 1)],
                        start=(d == 0), stop=(d == NMT - 1))
                nc.vector.tensor_copy(ot[:, 512 * n:512 * (n + 1)], po_ps[:])
            nc.sync.dma_start(out_ap[128 * qt:128 * (qt + 1), :], ot[:])

        # chunk-0 projections up front; later chunks' projection groups are
        # emitted as PE filler between attention heads
        def make_fillers(qc):
            """Closures emitting one PE group each for chunk qc's projections."""
            xq_c = load_xt_chunk(xq_ap, qc, "q")
            xk_c = load_xt_chunk(xk_ap, qc, "k")
            xv_c = load_xt_chunk(xv_ap, qc, "v")
            f = []
            for m in range(NMT):
                f.append(lambda m=m: proj_chunk(xq_c, wq_sb, bq_sb, qT, qc, m))
                f.append(lambda m=m: proj_chunk(xk_c, wk_sb, bk_sb, kT, qc, m))
            for sti in range(4):
                f.append(lambda sti=sti: v_chunk(xv_c, qc, sti))
            return f

        for flr in make_fillers(0):
            flr()

        for qc in range(NQC):
            fillers = list(make_fillers(qc + 1)) if qc + 1 < NQC else []
            if qc > 0:
                fillers += [lambda qt=4 * (qc - 1) + j: emit_outproj(qt)
                            for j in range(4)]
            nf = 0
            nkt = 4 * (qc + 1)  # causal: key tiles 0..nkt-1
            for hp in range(HL // 2):
                m = hp
                heads = (2 * hp, 2 * hp + 1)
                ctx_ps = {h: psum_ctx.tile([HD + 1, 512], F32, tag="ctx",
                                           name=f"ctx{h}") for h in heads}
                def emit_scores_exp(kt):
                    qs = max(0, 128 * kt - 512 * qc)  # local q start
                    sc_ps = psum_big.tile([128, 1024], F32, tag="big", name="sc")
                    # even/odd heads at partition bases 0/64: row-tiled, run
                    # concurrently in the PE array
                    for i, h in enumerate(heads):
                        po = 64 * i
                        nc.tensor.matmul(
                            sc_ps[:, 512 * i + qs:512 * (i + 1)],
                            kT[m][po:po + HD, 128 * kt:128 * (kt + 1)],
                            qT[m][po:po + HD, 512 * qc + qs:512 * (qc + 1)],
                            start=True, stop=True)
                    et = exp_pool.tile([128, 1024], BF16, tag="et", name="et")
                    nc.scalar.activation(et[:, qs:1024], sc_ps[:, qs:1024],
                                         EXP, scale=SCALE)
                    return et

                def emit_ctx(kt, et):
                    qs = max(0, 128 * kt - 512 * qc)
                    diag = 4 * qc <= kt < 4 * qc + 4
                    for i, h in enumerate(heads):
                        if diag:  # mask k>q in the diagonal 128x128 block
                            nc.gpsimd.affine_select(
                                out=et[:, 512 * i + qs:512 * i + qs + 128],
                                in_=et[:, 512 * i + qs:512 * i + qs + 128],
                                compare_op=mybir.AluOpType.is_ge, fill=0.0,
                                base=0, pattern=[[1, 128]], channel_multiplier=-1)
                        nc.tensor.matmul(
                            ctx_ps[h][:, qs:512],
                            v_aug[kt][:].rearrange("p (h c) -> p h c", h=HL)[:, h, :],
                            et[:, 512 * i + qs:512 * (i + 1)],
                            start=(kt == 0), stop=(kt == nkt - 1))

                # software pipeline: scores/exp run 2 iterations ahead of ctx
                pend = []
                for kt in range(nkt):
                    pend.append((kt, emit_scores_exp(kt)))
                    if len(pend) > 2:
                        emit_ctx(*pend.pop(0))
                    # PE filler spread across the whole chunk's kt iterations
                    want = (len(fillers) * (hp * nkt + kt + 1)) // (HL // 2 * nkt)
                    while nf < want:
                        fillers[nf]()
                        nf += 1
                for p in pend:
                    emit_ctx(*p)
                for h in heads:
                    # normalize into ctxT
                    po = 64 * (h % 2)
                    sums_sb = norm_pool.tile([1, 512], F32, tag="sums", name="sums")
                    nc.vector.tensor_copy(sums_sb[:], ctx_ps[h][HD:HD + 1, :])
                    recip = norm_pool.tile([1, 512], F32, tag="recip", name="recip")
                    nc.vector.reciprocal_approx_fast(recip[:], sums_sb[:])
                    rep = norm_pool.tile([HD, 512], F32, tag="rep", name="rep")
                    nc.gpsimd.partition_broadcast(rep[:], recip[:])
                    nc.vector.tensor_mul(
                        ctxT[m][po:po + HD, 512 * qc:512 * (qc + 1)],
                        ctx_ps[h][0:HD, :], rep[:])

        for qt in range(4 * (NQC - 1), 4 * NQC):
            emit_outproj(qt)

    nc.compile()
    return nc


def _shard(inputs):
    in_maps = []
    for c in range(N_CORES):
        b, g = c // 2, c % 2
        sl = slice(512 * g, 512 * (g + 1))
        in_maps.append({
            "xqt": np.ascontiguousarray(inputs["inputs_q"][b].T.astype(ml_dtypes.bfloat16)),
            "xkt": np.ascontiguousarray(inputs["inputs_k"][b].T.astype(ml_dtypes.bfloat16)),
            "xvt": np.ascontiguousarray(inputs["inputs_v"][b].T.astype(ml_dtypes.bfloat16)),
            "wq": np.ascontiguousarray(inputs["Wq"][:, sl].astype(ml_dtypes.bfloat16)),
            "wk": np.ascontiguousarray(inputs["Wk"][:, sl].astype(ml_dtypes.bfloat16)),
            "wv": np.ascontiguousarray(inputs["Wv"][:, sl].astype(ml_dtypes.bfloat16)),
            "bq": np.ascontiguousarray(inputs["bq"][sl])[:, None],
            "bk": np.ascontiguousarray(inputs["bk"][sl])[:, None],
            "bvb": np.ascontiguousarray(
                np.broadcast_to(inputs["bv"][sl], (128, 512))),
            "wo": np.ascontiguousarray(inputs["Wo"][sl, :].astype(ml_dtypes.bfloat16)),
        })
    return in_maps


def kernel(**inputs):
    global _compiled
    inputs = {k: np.asarray(v, dtype=np.float32) for k, v in inputs.items()}
    if _compiled is None:
        _compiled = _build()
    nc = _compiled
    in_maps = _shard(inputs)
    res = run_bass_kernel_spmd(nc, in_maps, list(range(N_CORES)),
                               trace=bool(int(__import__("os").environ.get("BASS_TRACE", "0"))))
    kernel.last_results = res
    B = 4
    out = np.empty((B, S, D), np.float32)
    for b in range(B):
        out[b] = res.results[2 * b]["out"] + res.results[2 * b + 1]["out"]
    out += inputs["bo"][None, None, :]
    return out


# revision 29
# speedup vs baseline: 1.0550x; 1.0550x over previous
"""Multi-head causal attention (B=4, S=2048, D=1024, H=16) on 8 TRN2 NeuronCores.

Sharding: core c -> (batch c//2, head-group c%2 of 8 heads = 512 d_model cols).
Each core:
  - projects Q/K/V for its head slice (bf16 matmuls, fp32 accum)
  - causal attention for its 8 heads over the full sequence, computed with
    scores transposed ([keys, q]) so exp(scores)^T feeds the A@V matmul as the
    moving operand; V is augmented with a ones column so softmax sums fall out
    of the same matmul
  - partial out-projection ctx^T @ Wo[rows-of-its-heads]  (no bias)
Host: out[b] = partial[2b] + partial[2b+1] + bo.
"""

import numpy as np
import ml_dtypes
from contextlib import ExitStack

import concourse.bass as bass
import concourse.tile as tile
from concourse import bacc, mybir
from concourse.bass_utils import run_bass_kernel_spmd

F32 = mybir.dt.float32
BF16 = mybir.dt.bfloat16
EXP = mybir.ActivationFunctionType.Exp

N_CORES = 8
S = 2048          # sequence length
D = 1024          # d_model
HL = 8            # heads per core
HD = 64           # head dim
DL = HL * HD      # local d_model slice = 512
SCALE = 1.0 / 8.0  # 1/sqrt(HD)

NQT = S // 128    # 16 q/seq tiles of 128
NQC = S // 512    # 4 q chunks of 512
NKT = S // 128    # 16 key tiles of 128
NDT = D // 128    # 8 d_model(in) tiles
NMT = DL // 128   # 4 local dout tiles (head pairs)
G = 2             # key-tiles per scores/exp group (2 PSUM banks)

_compiled = None  # cached (nc,) so repeated kernel() calls skip rebuild


def _build():
    nc = bacc.Bacc("TRN2", target_bir_lowering=False, debug=False,
                   num_devices=N_CORES)

    xq_ap = nc.dram_tensor("xqt", [D, S], BF16, kind="ExternalInput").ap()
    xk_ap = nc.dram_tensor("xkt", [D, S], BF16, kind="ExternalInput").ap()
    xv_ap = nc.dram_tensor("xvt", [D, S], BF16, kind="ExternalInput").ap()
    wq_ap = nc.dram_tensor("wq", [D, DL], BF16, kind="ExternalInput").ap()
    wk_ap = nc.dram_tensor("wk", [D, DL], BF16, kind="ExternalInput").ap()
    wv_ap = nc.dram_tensor("wv", [D, DL], BF16, kind="ExternalInput").ap()
    bq_ap = nc.dram_tensor("bq", [DL, 1], F32, kind="ExternalInput").ap()
    bk_ap = nc.dram_tensor("bk", [DL, 1], F32, kind="ExternalInput").ap()
    bvb_ap = nc.dram_tensor("bvb", [128, DL], F32, kind="ExternalInput").ap()
    wo_ap = nc.dram_tensor("wo", [DL, D], BF16, kind="ExternalInput").ap()
    out_ap = nc.dram_tensor("out", [S, D], F32, kind="ExternalOutput").ap()

    with tile.TileContext(nc) as tc, ExitStack() as ctx:
        wpool = ctx.enter_context(tc.tile_pool(name="weights", bufs=1))
        xt_pool = ctx.enter_context(tc.tile_pool(name="xt", bufs=64))
        qkv_pool = ctx.enter_context(tc.tile_pool(name="qkv", bufs=1))
        exp_pool = ctx.enter_context(tc.tile_pool(name="expt", bufs=4))
        norm_pool = ctx.enter_context(tc.tile_pool(name="norm", bufs=4))
        outst_pool = ctx.enter_context(tc.tile_pool(name="outst", bufs=2))
        psum_big = ctx.enter_context(tc.tile_pool(name="ps_big", bufs=3, space="PSUM"))
        psum_ctx = ctx.enter_context(tc.tile_pool(name="ps_ctx", bufs=2, space="PSUM"))

        # ---- weights / biases (already bf16 in DRAM) ----
        def load_w(dram, shape, nm):
            t16 = wpool.tile(shape, BF16, tag=nm, name=nm)
            nc.sync.dma_start(t16[:], dram)
            return t16

        wq_sb = [load_w(wq_ap[128 * d:128 * (d + 1), :], [128, DL], f"wq{d}") for d in range(NDT)]
        wk_sb = [load_w(wk_ap[128 * d:128 * (d + 1), :], [128, DL], f"wk{d}") for d in range(NDT)]
        wv_sb = [load_w(wv_ap[128 * d:128 * (d + 1), :], [128, DL], f"wv{d}") for d in range(NDT)]
        wo_sb = [load_w(wo_ap[128 * d:128 * (d + 1), :], [128, D], f"wo{d}") for d in range(NMT)]

        bq_sb = wpool.tile([128, NMT], F32, tag="bq")
        bk_sb = wpool.tile([128, NMT], F32, tag="bk")
        for m in range(NMT):
            nc.sync.dma_start(bq_sb[:, m:m + 1], bq_ap[128 * m:128 * (m + 1), :])
            nc.sync.dma_start(bk_sb[:, m:m + 1], bk_ap[128 * m:128 * (m + 1), :])
        bvb_sb = wpool.tile([128, DL], F32, tag="bvb")
        nc.sync.dma_start(bvb_sb[:], bvb_ap[:])

        # ---- x^T chunk load (host pre-transposed + pre-cast bf16) ----
        # 8 tiles [128 din, 512 seq] per (input, chunk)
        def load_xt_chunk(x_ap, qc, nm):
            xt = []
            for d in range(NDT):
                t = xt_pool.tile([128, 512], BF16, tag="xt", name=f"{nm}xt{qc}_{d}")
                nc.sync.dma_start(
                    t[:], x_ap[128 * d:128 * (d + 1), 512 * qc:512 * (qc + 1)])
                xt.append(t)
            return xt

        # qT/kT: [DL, S] bf16 stored as NMT tiles [128, S]
        qT = [qkv_pool.tile([128, S], BF16, tag=f"qT{m}", name=f"qT{m}") for m in range(NMT)]
        kT = [qkv_pool.tile([128, S], BF16, tag=f"kT{m}", name=f"kT{m}") for m in range(NMT)]

        def proj_chunk(xt, w_sb, b_sb, res, qc, m):
            ps = psum_big.tile([128, 512], F32, tag="big", name="ps")
            for d in range(NDT):
                nc.tensor.matmul(
                    ps[:], w_sb[d][:, 128 * m:128 * (m + 1)],
                    xt[d][:],
                    start=(d == 0), stop=(d == NDT - 1))
            nc.vector.tensor_scalar_add(
                res[m][:, 512 * qc:512 * (qc + 1)], ps[:],
                b_sb[:, m:m + 1])

        # v_aug: per seq-tile [128, HL*(HD+1)] bf16; per head 64 v cols + ones col
        v_aug = [None] * NQT

        def v_chunk(xt, qc, sti):
            st = 4 * qc + sti
            va = qkv_pool.tile([128, HL * (HD + 1)], BF16, tag=f"va{st}",
                               name=f"va{st}")
            nc.vector.memset(va[:], 1.0)
            ps = psum_big.tile([128, DL], F32, tag="big", name="ps")
            for d in range(NDT):
                nc.tensor.matmul(ps[:], xt[d][:, 128 * sti:128 * (sti + 1)],
                                 wv_sb[d][:], start=(d == 0), stop=(d == NDT - 1))
            va3 = va[:].rearrange("p (h c) -> p h c", h=HL)[:, :, 0:HD]
            nc.vector.tensor_add(
                va3,
                ps[:].rearrange("p (h c) -> p h c", h=HL),
                bvb_sb[:].rearrange("p (h c) -> p h c", h=HL))
            v_aug[st] = va

        # ---- attention + out projection, per q-chunk ----
        # ctxT: per head-pair tile [128, S] bf16 (rows 64*(h%2) for head h)
        ctxT = [qkv_pool.tile([128, S], BF16, tag=f"ctxT{m}", name=f"ctxT{m}") for m in range(NMT)]

        def emit_outproj(qt):
            ot = outst_pool.tile([128, 1024], F32, tag="ot", name="ot")
            for n in range(2):
                po_ps = psum_big.tile([128, 512], F32, tag="big", name="po_ps")
                for d in range(NMT):
                    nc.tensor.matmul(
                        po_ps[:],
                        ctxT[d][:, 128 * qt:128 * (qt + 1)],
                        wo_sb[d][:, 512 * n:512 * (n + 1)],
                        start=(d == 0), stop=(d == NMT - 1))
                nc.vector.tensor_copy(ot[:, 512 * n:512 * (n + 1)], po_ps[:])
            nc.sync.dma_start(out_ap[128 * qt:128 * (qt + 1), :], ot[:])

        # chunk-0 projections up front; later chunks' projection groups are
        # emitted as PE filler between attention heads
        def make_fillers(qc):
            """Closures emitting one PE group each for chunk qc's projections."""
            xq_c = load_xt_chunk(xq_ap, qc, "q")
            xk_c = load_xt_chunk(xk_ap, qc, "k")
            xv_c = load_xt_chunk(xv_ap, qc, "v")
            f = []
            for m in range(NMT):
                f.append(lambda m=m: proj_chunk(xq_c, wq_sb, bq_sb, qT, qc, m))
                f.append(lambda m=m: proj_chunk(xk_c, wk_sb, bk_sb, kT, qc, m))
            for sti in range(4):
                f.append(lambda sti=sti: v_chunk(xv_c, qc, sti))
            return f

        for flr in make_fillers(0):
            flr()

        for qc in range(NQC):
            fillers = list(make_fillers(qc + 1)) if qc + 1 < NQC else []
            if qc > 0:
                fillers += [lambda qt=4 * (qc - 1) + j: emit_outproj(qt)
                            for j in range(4)]
            nf = 0
            nkt = 4 * (qc + 1)  # causal: key tiles 0..nkt-1
            for hp in range(HL // 2):
                m = hp
                heads = (2 * hp, 2 * hp + 1)
                ctx_ps = {h: psum_ctx.tile([HD + 1, 512], F32, tag="ctx",
                                           name=f"ctx{h}") for h in heads}
                def emit_scores_exp(kt):
                    qs = max(0, 128 * kt - 512 * qc)  # local q start
                    sc_ps = psum_big.tile([128, 1024], F32, tag="big", name="sc")
                    # even/odd heads at partition bases 0/64: row-tiled, run
                    # concurrently in the PE array
                    for i, h in enumerate(heads):
                        po = 64 * i
                        nc.tensor.matmul(
                            sc_ps[:, 512 * i + qs:512 * (i + 1)],
                            kT[m][po:po + HD, 128 * kt:128 * (kt + 1)],
                            qT[m][po:po + HD, 512 * qc + qs:512 * (qc + 1)],
                            start=True, stop=True)
                    et = exp_pool.tile([128, 1024], BF16, tag="et", name="et")
                    nc.scalar.activation(et[:, qs:1024], sc_ps[:, qs:1024],
                                         EXP, scale=SCALE)
                    return et

                def emit_ctx(kt, et):
                    qs = max(0, 128 * kt - 512 * qc)
                    diag = 4 * qc <= kt < 4 * qc + 4
                    for i, h in enumerate(heads):
                        if diag:  # mask k>q in the diagonal 128x128 block
                            nc.gpsimd.affine_select(
                                out=et[:, 512 * i + qs:512 * i + qs + 128],
                                in_=et[:, 512 * i + qs:512 * i + qs + 128],
                                compare_op=mybir.AluOpType.is_ge, fill=0.0,
                                base=0, pattern=[[1, 128]], channel_multiplier=-1)
                        nc.tensor.matmul(
                            ctx_ps[h][:, qs:512],
                            v_aug[kt][:].rearrange("p (h c) -> p h c", h=HL)[:, h, :],
                            et[:, 512 * i + qs:512 * (i + 1)],
                            start=(kt == 0), stop=(kt == nkt - 1))

                # software pipeline: scores/exp run 2 iterations ahead of ctx
                pend = []
                for kt in range(nkt):
                    pend.append((kt, emit_scores_exp(kt)))
                    if len(pend) > 2:
                        emit_ctx(*pend.pop(0))
                    # PE filler spread across the whole chunk's kt iterations
                    want = (len(fillers) * (hp * nkt + kt + 1)) // (HL // 2 * nkt)
                    while nf < want:
                        fillers[nf]()
                        nf += 1
                for p in pend:
                    emit_ctx(*p)
                for h in heads:
                    # evacuate psum fast (frees the ctx slot), then normalize
                    po = 64 * (h % 2)
                    cu = norm_pool.tile([HD + 1, 512], F32, tag="cu", name="cu")
                    nc.vector.tensor_copy(cu[:], ctx_ps[h][:])
                    recip = norm_pool.tile([1, 512], F32, tag="recip", name="recip")
                    nc.vector.reciprocal_approx_fast(recip[:], cu[HD:HD + 1, :])
                    rep = norm_pool.tile([HD, 512], F32, tag="rep", name="rep")
                    nc.gpsimd.partition_broadcast(rep[:], recip[:])
                    nc.vector.tensor_mul(
                        ctxT[m][po:po + HD, 512 * qc:512 * (qc + 1)],
                        cu[0:HD, :], rep[:])

        for qt in range(4 * (NQC - 1), 4 * NQC):
            emit_outproj(qt)

    nc.compile()
    return nc


def _shard(inputs):
    in_maps = []
    for c in range(N_CORES):
        b, g = c // 2, c % 2
        sl = slice(512 * g, 512 * (g + 1))
        in_maps.append({
            "xqt": np.ascontiguousarray(inputs["inputs_q"][b].T.astype(ml_dtypes.bfloat16)),
            "xkt": np.ascontiguousarray(inputs["inputs_k"][b].T.astype(ml_dtypes.bfloat16)),
            "xvt": np.ascontiguousarray(inputs["inputs_v"][b].T.astype(ml_dtypes.bfloat16)),
            "wq": np.ascontiguousarray(inputs["Wq"][:, sl].astype(ml_dtypes.bfloat16)),
            "wk": np.ascontiguousarray(inputs["Wk"][:, sl].astype(ml_dtypes.bfloat16)),
            "wv": np.ascontiguousarray(inputs["Wv"][:, sl].astype(ml_dtypes.bfloat16)),
            "bq": np.ascontiguousarray(inputs["bq"][sl])[:, None],
            "bk": np.ascontiguousarray(inputs["bk"][sl])[:, None],
            "bvb": np.ascontiguousarray(
                np.broadcast_to(inputs["bv"][sl], (128, 512))),
            "wo": np.ascontiguousarray(inputs["Wo"][sl, :].astype(ml_dtypes.bfloat16)),
        })
    return in_maps


def kernel(**inputs):
    global _compiled
    inputs = {k: np.asarray(v, dtype=np.float32) for k, v in inputs.items()}
    if _compiled is None:
        _compiled = _build()
    nc = _compiled
    in_maps = _shard(inputs)
    res = run_bass_kernel_spmd(nc, in_maps, list(range(N_CORES)),
                               trace=bool(int(__import__("os").environ.get("BASS_TRACE", "0"))))
    kernel.last_results = res
    B = 4
    out = np.empty((B, S, D), np.float32)
    for b in range(B):
        out[b] = res.results[2 * b]["out"] + res.results[2 * b + 1]["out"]
    out += inputs["bo"][None, None, :]
    return out
